# revision 36
# baseline (speedup 1.0000x reference)
"""Multi-head attention (B=4, L=1024, D=1024, H=16, DH=64) on 8 TRN2 NeuronCores.

Sharding: data-parallel over batch (4) x tensor-parallel over heads (2).
Core c = 2*b + t computes, for batch b, heads [t*8, (t+1)*8):
    QT = Wq_t^T X^T, KT = Wk_t^T X^T, V = Y Wv_t        (all bf16 matmuls)
    per head: S^T = K_h Q_h^T; P^T = exp(S^T/8);
              [ctx^T; rowsum] = Vaug_h^T P^T;  ctxn = ctx / rowsum
    O_partial = ctxn^T Wo_t                              (f32, two dt-halves)
Host pre-transposes X/Y, casts to bf16, and sums the four f32 partials
(2 tensor-parallel cores x 2 dt-halves) per batch.

Engines execute their compiled instruction streams in order, so the emission
order is a hand-software-pipelined schedule: every ST (scores) step, whose exp
drain on the scalar engine is 2.6x slower than the matmuls, is followed by an
independent ~8-matmul "fill chain" (V projection, next d-tile QT/KT, an earlier
head's ctx, or an out-projection partial) so the tensor engine never waits for
the scalar engine to free an ST PSUM tile.
"""

import numpy as np
import ml_dtypes

import concourse.tile as tile
import concourse.mybir as mybir
from concourse import bacc
from concourse.bass_utils import run_bass_kernel_spmd

B, L, D, U, H = 4, 1024, 1024, 1024, 16
DH = U // H          # 64 head dim
TP = 2               # tensor-parallel ways (heads)
DL = U // TP         # 512 local units
HL = H // TP         # 8 local heads
P = 128              # partitions
NI = 512             # matmul free-dim chunk (one PSUM bank of f32)
CC = D // P          # 8 contraction chunks for projections
DT = DL // P         # 4 local d-tiles
IT = L // P          # 8 i/j tiles
NIC = L // NI        # 2 free chunks of 512
N_CORES = 8

BF16 = mybir.dt.bfloat16
F32 = mybir.dt.float32


def _build_kernel():
    nc = bacc.Bacc(
        "TRN2", target_bir_lowering=False, debug=False, num_devices=N_CORES
    )
    xt = nc.dram_tensor("xt", [D, L], BF16, kind="ExternalInput").ap()
    yt = nc.dram_tensor("yt", [D, L], BF16, kind="ExternalInput").ap()
    wq = nc.dram_tensor("wq", [D, DL], BF16, kind="ExternalInput").ap()
    wk = nc.dram_tensor("wk", [D, DL], BF16, kind="ExternalInput").ap()
    wv = nc.dram_tensor("wv", [D, DL], BF16, kind="ExternalInput").ap()
    wo = nc.dram_tensor("wo", [DL, U], BF16, kind="ExternalInput").ap()
    out_a = nc.dram_tensor("out_a", [L, U], BF16, kind="ExternalOutput").ap()
    out_b = nc.dram_tensor("out_b", [L, U], BF16, kind="ExternalOutput").ap()
    out_c = nc.dram_tensor("out_c", [L, U], BF16, kind="ExternalOutput").ap()

    with tile.TileContext(nc) as tc:
        _mha_body(tc, out_a, out_b, out_c, xt, yt, wq, wk, wv, wo)

    nc.compile()
    return nc


def _mha_body(tc, out_a, out_b, out_c, xt, yt, wq, wk, wv, wo, dbg=None):
    nc = tc.nc
    from contextlib import ExitStack

    with ExitStack() as ctx:
        persist = ctx.enter_context(tc.tile_pool(name="persist", bufs=1))
        pt_pool = ctx.enter_context(tc.tile_pool(name="pt", bufs=4))
        # ST tiles: [P, 1024] f32 = 2 banks each
        ps_wide = ctx.enter_context(tc.tile_pool(name="ps_wide", bufs=2, space="PSUM"))
        # single-bank accumulators (projections, V, ctx, out-proj)
        ps_acc = ctx.enter_context(tc.tile_pool(name="ps_acc", bufs=4, space="PSUM"))
        small = ctx.enter_context(tc.tile_pool(name="small", bufs=4))

        # persistent SBUF tensors
        xt_sb = persist.tile([P, CC, L], BF16, tag="xt")
        yt_sb = persist.tile([P, CC, L], BF16, tag="yt")
        wq_sb = persist.tile([P, CC, DL], BF16, tag="wq")
        wk_sb = persist.tile([P, CC, DL], BF16, tag="wk")
        wv_sb = persist.tile([P, CC, DL], BF16, tag="wv")
        wo_sb = persist.tile([P, DT, U], BF16, tag="wo")
        qt_sb = persist.tile([P, DT, L], BF16, tag="qt")
        kt_sb = persist.tile([P, DT, L], BF16, tag="kt")
        # Vaug: per j-chunk, per head a 128-col block; even h: [V_h | ones],
        # odd h: [ones | V_h] (ctx^T lands on the head's own cx partitions)
        va_sb = persist.tile([P, IT, HL * P], BF16, tag="va")
        cx_sb = persist.tile([P, DT, L], BF16, tag="cx")

        # chunked input DMAs, priority-ordered so the first QT0/KT0 ic0
        # chains (and head-pair-0 scores) can start as early as possible
        wq_r = wq.rearrange("(cc p) d -> p cc d", p=P)
        wk_r = wk.rearrange("(cc p) d -> p cc d", p=P)
        wv_r = wv.rearrange("(cc p) d -> p cc d", p=P)
        xt_r = xt.rearrange("(cc p) i -> p cc i", p=P)
        yt_r = yt.rearrange("(cc p) i -> p cc i", p=P)
        for cc in range(CC):
            nc.sync.dma_start(out=wq_sb[:, cc], in_=wq_r[:, cc])
            nc.sync.dma_start(out=xt_sb[:, cc], in_=xt_r[:, cc])
        for cc in range(CC):
            nc.sync.dma_start(out=wk_sb[:, cc], in_=wk_r[:, cc])
            nc.sync.dma_start(out=yt_sb[:, cc], in_=yt_r[:, cc])
        for cc in range(CC):
            nc.sync.dma_start(out=wv_sb[:, cc], in_=wv_r[:, cc])
        nc.vector.memset(va_sb[:], 1.0)
        nc.sync.dma_start(out=wo_sb[:], in_=wo.rearrange("(dt p) o -> p dt o", p=P))

        scale = DH**-0.5

        # ---- chain emitters (each a short burst of independent PE work) ----

        def proj_chain(w_sb, t_sb, rhs_sb, dt, ic):
            ps = ps_acc.tile([P, NI], F32, tag="acc")
            for cc in range(CC):
                nc.tensor.matmul(
                    ps[:],
                    w_sb[:, cc, dt * P : (dt + 1) * P],
                    rhs_sb[:, cc, ic * NI : (ic + 1) * NI],
                    start=(cc == 0),
                    stop=(cc == CC - 1),
                )
            nc.vector.tensor_copy(t_sb[:, dt, ic * NI : (ic + 1) * NI], ps[:])

        def v_chain(jt):
            ps = ps_acc.tile([P, NI], F32, tag="acc")
            for cc in range(CC):
                nc.tensor.matmul(
                    ps[:],
                    yt_sb[:, cc, jt * P : (jt + 1) * P],
                    wv_sb[:, cc, :],
                    start=(cc == 0),
                    stop=(cc == CC - 1),
                )
            va_blk = va_sb[:, jt].rearrange("p (h s) -> p h s", s=P)
            ps_blk = ps.rearrange("p (h s) -> p h s", s=DH)
            nc.vector.tensor_copy(va_blk[:, 0::2, 0:DH], ps_blk[:, 0::2, :])
            nc.vector.tensor_copy(va_blk[:, 1::2, DH:P], ps_blk[:, 1::2, :])

        # Deferred finishers: the normalize needs a SBUF->SBUF DMA between
        # two DVE ops; emitting the post-DMA ops immediately would stall the
        # in-order DVE stream (and the PSUM-releasing copies queued behind
        # it) for the DMA round-trip. Instead each ctx chain queues its
        # post-DMA ops and the next fill slot flushes them.
        deferred = []

        def flush_deferred():
            while deferred:
                deferred.pop(0)()

        def ctx_chain(h, ptile, ic):
            dt, r0 = divmod(h * DH, P)
            ct = ps_acc.tile([P, NI], F32, tag="acc")
            for jt in range(IT):
                nc.tensor.matmul(
                    ct[:],
                    va_sb[:, jt, h * P : (h + 1) * P],
                    ptile[:, jt, ic * NI : (ic + 1) * NI],
                    start=(jt == 0),
                    stop=(jt == IT - 1),
                )
            # custom DVE ops (reciprocal) only work at base partition 0;
            # cross-partition moves go through small SBUF->SBUF DMAs.
            rc = small.tile([P, NI], F32, tag="rc")
            if r0 == 0:
                rs = small.tile([P, NI], F32, tag="rs")
                nc.vector.tensor_copy(rs[DH:P, :], ct[DH:P, :])
                nc.gpsimd.dma_start(out=rs[0:DH, :], in_=rs[DH:P, :])

                def fin():
                    nc.vector.reciprocal_approx_fast(rc[0:DH, :], rs[0:DH, :])
                    nc.vector.tensor_mul(
                        cx_sb[0:DH, dt, ic * NI : (ic + 1) * NI],
                        ct[0:DH, :],
                        rc[0:DH, :],
                    )
            else:
                nc.vector.reciprocal_approx_fast(rc[0:DH, :], ct[0:DH, :])
                nc.gpsimd.dma_start(out=rc[DH:P, :], in_=rc[0:DH, :])

                def fin():
                    nc.vector.tensor_mul(
                        cx_sb[DH:P, dt, ic * NI : (ic + 1) * NI],
                        ct[DH:P, :],
                        rc[DH:P, :],
                    )

            deferred.append(fin)

        def po_wide_chain(it, dts, out_ap):
            # both oc halves in one 2-bank psum tile (shares the ST pool
            # slots, which are idle once the score phases are done); single
            # FD=1024 bf16 drain + one 256KB DMA
            po = ps_wide.tile([P, 2 * NI], F32, tag="wide")
            for oc in range(NIC):
                for k, dt in enumerate(dts):
                    nc.tensor.matmul(
                        po[:, oc * NI : (oc + 1) * NI],
                        cx_sb[:, dt, it * P : (it + 1) * P],
                        wo_sb[:, dt, oc * NI : (oc + 1) * NI],
                        start=(k == 0),
                        stop=(k == len(dts) - 1),
                    )
            o_st = small.tile([P, 2 * NI], BF16, tag="ostw")
            nc.vector.tensor_copy(o_st[:], po[:])
            out_r = out_ap.rearrange("(it p) o -> it p o", p=P)
            nc.sync.dma_start(out=out_r[it], in_=o_st[:])

        def po_chain(it, oc, dts, out_ap, copy_eng="vector"):
            # out-projection partial over the given d-tiles
            po = ps_acc.tile([P, NI], F32, tag="acc")
            for k, dt in enumerate(dts):
                nc.tensor.matmul(
                    po[:],
                    cx_sb[:, dt, it * P : (it + 1) * P],
                    wo_sb[:, dt, oc * NI : (oc + 1) * NI],
                    start=(k == 0),
                    stop=(k == len(dts) - 1),
                )
            o_st = small.tile([P, NI], BF16, tag="ost")
            if copy_eng == "vector":
                nc.vector.tensor_copy(o_st[:], po[:])
            else:
                # scalar engine is idle once the exp stream has drained
                nc.scalar.copy(o_st[:], po[:])
            out_r = out_ap.rearrange("(it p) o -> it p o", p=P)
            nc.sync.dma_start(
                out=out_r[it, :, oc * NI : (oc + 1) * NI], in_=o_st[:]
            )

        # ---- ST + exp for a head pair, fill chains between steps ----

        def st_pair(hp, fills):
            dt = hp
            ptiles = []
            for h_off in range(2):
                pt_tile = pt_pool.tile([P, IT, L], BF16, tag="pt")
                ptiles.append(pt_tile)
            fills = list(fills)
            for jt in range(IT):
                sts = []
                for h_off in range(2):
                    r0 = DH * h_off
                    st = ps_wide.tile([P, 2 * NI], F32, tag="wide")
                    sts.append(st)
                    for ic in range(NIC):
                        nc.tensor.matmul(
                            st[:, ic * NI : (ic + 1) * NI],
                            kt_sb[r0 : r0 + DH, dt, jt * P : (jt + 1) * P],
                            qt_sb[r0 : r0 + DH, dt, ic * NI : (ic + 1) * NI],
                            start=True,
                            stop=True,
                        )
                for h_off in range(2):
                    nc.scalar.activation(
                        ptiles[h_off][:, jt, :],
                        sts[h_off][:],
                        mybir.ActivationFunctionType.Exp,
                        scale=scale,
                    )
                if jt < len(fills):
                    pending = list(deferred)
                    deferred.clear()
                    for f in fills[jt]:
                        f()
                    for f in pending:
                        f()
            return ptiles

        # ---- schedule ----
        mk = lambda f, *a: (lambda: f(*a))

        for ic in range(NIC):
            proj_chain(wq_sb, qt_sb, xt_sb, 0, ic)
        for ic in range(NIC):
            proj_chain(wk_sb, kt_sb, yt_sb, 0, ic)

        # pair 0: fill with the 8 V chains
        pt0 = st_pair(0, [[mk(v_chain, jt)] for jt in range(IT)])

        if dbg is not None:
            nc.sync.dma_start(out=dbg[3][0], in_=pt0[0][:])
            nc.sync.dma_start(out=dbg[3][1], in_=pt0[1][:])

        # QT1/KT1 ahead of pair 1 (also covers pair-0 exp tail)
        for ic in range(NIC):
            proj_chain(wq_sb, qt_sb, xt_sb, 1, ic)
        for ic in range(NIC):
            proj_chain(wk_sb, kt_sb, yt_sb, 1, ic)

        # pair 1: fill with ctx of heads 0/1 and QT2/KT2
        pt1 = st_pair(
            1,
            [
                [mk(ctx_chain, 0, pt0[0], 0)],
                [mk(ctx_chain, 0, pt0[0], 1)],
                [mk(ctx_chain, 1, pt0[1], 0)],
                [mk(ctx_chain, 1, pt0[1], 1)],
                [mk(proj_chain, wq_sb, qt_sb, xt_sb, 2, 0)],
                [mk(proj_chain, wq_sb, qt_sb, xt_sb, 2, 1)],
                [mk(proj_chain, wk_sb, kt_sb, yt_sb, 2, 0)],
                [mk(proj_chain, wk_sb, kt_sb, yt_sb, 2, 1)],
            ],
        )

        # pair 2: fill with ctx of heads 2/3 and QT3/KT3
        pt2 = st_pair(
            2,
            [
                [mk(ctx_chain, 2, pt1[0], 0)],
                [mk(ctx_chain, 2, pt1[0], 1)],
                [mk(ctx_chain, 3, pt1[1], 0)],
                [mk(ctx_chain, 3, pt1[1], 1)],
                [mk(proj_chain, wq_sb, qt_sb, xt_sb, 3, 0)],
                [mk(proj_chain, wq_sb, qt_sb, xt_sb, 3, 1)],
                [mk(proj_chain, wk_sb, kt_sb, yt_sb, 3, 0)],
                [mk(proj_chain, wk_sb, kt_sb, yt_sb, 3, 1)],
            ],
        )

        # pair 3: fill with ctx of heads 4/5 and out-proj partial A (dt 0/1,
        # which only needs heads 0..3); four 2-MM po chains per slot
        poA = [
            mk(po_chain, it, oc, (0, 1), out_a)
            for it in range(IT)
            for oc in range(NIC)
        ]
        pt3 = st_pair(
            3,
            [
                [mk(ctx_chain, 4, pt2[0], 0)],
                [mk(ctx_chain, 4, pt2[0], 1)],
                [mk(ctx_chain, 5, pt2[1], 0)],
                [mk(ctx_chain, 5, pt2[1], 1)],
                poA[0:4],
                poA[4:8],
                poA[8:12],
                poA[12:16],
            ],
        )

        # tail: ctx of heads 6/7 interleaved with out-proj partial B (dt 2,
        # which only needs heads 4/5), then partial C (dt 3) last
        poB = [mk(po_wide_chain, it, (2,), out_b) for it in range(IT)]
        tail_ctx = [
            mk(ctx_chain, 6, pt3[0], 0),
            mk(ctx_chain, 6, pt3[0], 1),
            mk(ctx_chain, 7, pt3[1], 0),
            mk(ctx_chain, 7, pt3[1], 1),
        ]
        for k in range(4):
            pending = list(deferred)
            deferred.clear()
            tail_ctx[k]()
            for f in poB[2 * k : 2 * (k + 1)]:
                f()
            for f in pending:
                f()
        flush_deferred()
        for it in range(IT):
            po_wide_chain(it, (3,), out_c)

        if dbg is not None:
            nc.sync.dma_start(out=dbg[0][:], in_=qt_sb[:])
            nc.sync.dma_start(out=dbg[1][:], in_=kt_sb[:])
            nc.sync.dma_start(out=dbg[2][:], in_=va_sb[:])
            nc.sync.dma_start(out=dbg[4][:], in_=cx_sb[:])


_NC_CACHE = None


def _get_nc():
    global _NC_CACHE
    if _NC_CACHE is None:
        _NC_CACHE = _build_kernel()
    return _NC_CACHE


def kernel(x, y, Wq, Wk, Wv, Wo, _trace=False):
    bf = ml_dtypes.bfloat16
    x = np.asarray(x, np.float32)
    y = np.asarray(y, np.float32)
    xtb = [np.ascontiguousarray(np.asarray(x[b]).T).astype(bf) for b in range(B)]
    ytb = [np.ascontiguousarray(np.asarray(y[b]).T).astype(bf) for b in range(B)]
    wqs = [np.ascontiguousarray(np.asarray(Wq)[:, t * DL : (t + 1) * DL]).astype(bf) for t in range(TP)]
    wks = [np.ascontiguousarray(np.asarray(Wk)[:, t * DL : (t + 1) * DL]).astype(bf) for t in range(TP)]
    wvs = [np.ascontiguousarray(np.asarray(Wv)[:, t * DL : (t + 1) * DL]).astype(bf) for t in range(TP)]
    wos = [np.ascontiguousarray(np.asarray(Wo)[t * DL : (t + 1) * DL, :]).astype(bf) for t in range(TP)]

    in_maps = []
    for b in range(B):
        for t in range(TP):
            in_maps.append(
                {
                    "xt": xtb[b],
                    "yt": ytb[b],
                    "wq": wqs[t],
                    "wk": wks[t],
                    "wv": wvs[t],
                    "wo": wos[t],
                }
            )

    nc = _get_nc()
    res = run_bass_kernel_spmd(
        nc, in_maps, core_ids=list(range(N_CORES)), trace=_trace
    )
    out = np.empty((B, L, U), np.float32)
    for b in range(B):
        out[b] = (
            np.asarray(res.results[2 * b]["out_a"], np.float32)
            + np.asarray(res.results[2 * b]["out_b"], np.float32)
            + np.asarray(res.results[2 * b]["out_c"], np.float32)
            + np.asarray(res.results[2 * b + 1]["out_a"], np.float32)
            + np.asarray(res.results[2 * b + 1]["out_b"], np.float32)
            + np.asarray(res.results[2 * b + 1]["out_c"], np.float32)
        )
    if _trace:
        return out, res
    return out
